# revision 1
# baseline (speedup 1.0000x reference)
"""GCN message-passing kernel for Trainium2, n-core SPMD.

Pipeline (per core, SPMD identical program; per-core behavior comes from data):
  E-stage : hv0 = emb0[feat0]+emb1[feat1] (dma_gather from table); g0 = (srcn*hv0)@W0
  A-layer l (l=0..NL-1): per dst-block: agg = sum_{e: dst in block} g_l[src[e]]
            via dma_gather (pull rows) + one-hot matmul (segment sum on PE),
            hv_{l+1} = relu(dstn*agg + b_l); l<NL-1: g_{l+1} = (srcn*hv_{l+1})@W_{l+1}
  AllGather of g shards between layers (collective_compute).
  P-stage : group-pool (one-hot matmul, orientation [D x slots]), H-head attention
            on [G_core*L] slots, masked sum, whole-graph mean pool (rides layer-(NL-1)
            matmuls + the hv3 allgather), final linear -> [G_core, OUT] per core.

All per-(block,window) gather slots are padded to static capacities (max over
cores) so the SPMD instruction stream is core-invariant; pad slots use idx=0 with
the one-hot row disabled (dst_rel=-1 never equals iota 0..127).
"""

import math
from dataclasses import dataclass

import numpy as np

import concourse.bass as bass
import concourse.tile as tile
from concourse import bacc, mybir
from concourse.masks import make_identity

F16 = mybir.dt.float16
F32 = mybir.dt.float32
I16 = mybir.dt.int16
P = 128


@dataclass
class Cfg:
    N: int = 100_000
    E: int = 1_600_000
    B: int = 64
    L: int = 32
    D: int = 128
    V: int = 100
    H: int = 4
    OUT: int = 14
    NL: int = 3
    n_cores: int = 8
    window: int = 32768
    SB: int = 4          # dst blocks per gather super-block

    def __post_init__(self):
        assert self.D == P and self.H * self.L == P
        self.shard_blocks = math.ceil(math.ceil(self.N / self.n_cores) / P)
        self.shard = self.shard_blocks * P
        self.n_sb = math.ceil(self.shard_blocks / self.SB)
        # phase split (pipelined allgather): phase 0 = first half of the sbs
        self.sb_ph0 = self.n_sb // 2
        self.blk_ph0 = min(self.sb_ph0 * self.SB, self.shard_blocks)
        self.ph_rows = [self.blk_ph0 * P,
                        (self.shard_blocks - self.blk_ph0) * P + P]
        self.ph_N = [r * self.n_cores for r in self.ph_rows]
        self.n_win_p = [math.ceil(n / self.window) for n in self.ph_N]
        self.G_core = self.B // self.n_cores
        self.SLB = (self.B * self.L // self.n_cores) // P   # slot blocks per core
        assert self.SLB * P == self.G_core * self.L
        assert self.SLB <= self.SB
        self.dh = self.D // self.H

    def node_phase_row(self, n):
        """node id -> (phase, row in that phase's gathered tensor)."""
        n = np.asarray(n)
        c, loc = n // self.shard, n % self.shard
        ph = (loc >= self.ph_rows[0]).astype(np.int64)
        row = np.where(ph == 0, c * self.ph_rows[0] + loc,
                       c * self.ph_rows[1] + loc - self.ph_rows[0])
        return ph, row


def _wrap_idx(idx):
    """dma_gather index layout, unreplicated: [16, n/16] with t[p, s] =
    idx[s*16 + p].  The kernel replicates to 128 partitions during the
    SBUF load with a step-0 broadcast DMA (saves 8x on input upload)."""
    return np.ascontiguousarray(idx.reshape(-1, 16).T.astype(np.int16))


def _colmajor_chunks(vals, ncol):
    """[ncol*128] -> [128, ncol] with tile[p, c] = vals[c*128 + p]."""
    return np.ascontiguousarray(vals.reshape(ncol, P).T)


def _group_edges(rel, drel, blk, n_blocks):
    """Group window-filtered edges by block; dict blk -> (rel, drel)."""
    out = {}
    order = np.argsort(blk, kind="stable")
    rel, drel, blk = rel[order], drel[order], blk[order]
    bounds = np.searchsorted(blk, np.arange(n_blocks + 1))
    for b in range(n_blocks):
        lo, hi = bounds[b], bounds[b + 1]
        if hi > lo:
            out[b] = (rel[lo:hi], drel[lo:hi])
    return out


def preprocess(cfg: Cfg, inputs):
    f0 = np.asarray(inputs["feat0"]).astype(np.int64)
    f1 = np.asarray(inputs["feat1"]).astype(np.int64)
    src = np.asarray(inputs["src"]).astype(np.int64)
    dst = np.asarray(inputs["dst"]).astype(np.int64)
    graph_id = np.asarray(inputs["graph_id"]).astype(np.int64)
    gni = np.asarray(inputs["group_node_idx"]).astype(np.int64)
    gsi = np.asarray(inputs["group_seg_id"]).astype(np.int64)
    emb0 = np.asarray(inputs["emb0"], np.float32)
    emb1 = np.asarray(inputs["emb1"], np.float32)
    gcn_w = np.asarray(inputs["gcn_w"], np.float32)
    gcn_b = np.asarray(inputs["gcn_b"], np.float32)
    ipw = np.asarray(inputs["in_proj_w"], np.float32)
    ipb = np.asarray(inputs["in_proj_b"], np.float32)
    opw = np.asarray(inputs["out_proj_w"], np.float32)
    opb = np.asarray(inputs["out_proj_b"], np.float32)
    out_w = np.asarray(inputs["out_w"], np.float32)
    out_b = np.asarray(inputs["out_b"], np.float32)

    N, ncore, shard, sb_n = cfg.N, cfg.n_cores, cfg.shard, cfg.shard_blocks
    out_deg = np.maximum(np.bincount(src, minlength=N), 1.0)
    in_deg = np.maximum(np.bincount(dst, minlength=N), 1.0)
    srcn = (out_deg ** -0.5).astype(np.float32)
    dstn = (in_deg ** -0.5).astype(np.float32)
    srcn_p = np.ones(shard * ncore, np.float32)
    dstn_p = np.ones(shard * ncore, np.float32)
    srcn_p[:N], dstn_p[:N] = srcn, dstn

    src_ph, src_rows = cfg.node_phase_row(src)
    src_w = src_rows // cfg.window
    src_rel = src_rows - src_w * cfg.window
    core_of = dst // shard
    # (phase, window-in-phase) flat list shared by host and codegen
    pw_list = [(p, w) for p in range(2) for w in range(cfg.n_win_p[p])]
    n_pw = len(pw_list)

    groups = []
    for c in range(ncore):
        m = core_of == c
        s_p, s_w, s_rel = src_ph[m], src_w[m], src_rel[m]
        dl = dst[m] - c * shard
        per_w = []
        for (p, w) in pw_list:
            wm = (s_w == w) & (s_p == p)
            per_w.append(_group_edges(s_rel[wm], (dl % P)[wm], (dl // P)[wm], sb_n))
        groups.append(per_w)

    cap = np.zeros((sb_n, n_pw), np.int64)
    for c in range(ncore):
        for w in range(n_pw):
            for b, (r, _) in groups[c][w].items():
                cap[b, w] = max(cap[b, w], math.ceil(len(r) / P))

    # chunk schedule, shared between host arrays and codegen
    sched = []   # indexed [sb*n_pw + pw]
    col = 0
    for s in range(cfg.n_sb):
        blocks = range(s * cfg.SB, min((s + 1) * cfg.SB, sb_n))
        for w in range(n_pw):
            blks = [(b, int(cap[b, w])) for b in blocks if cap[b, w] > 0]
            nch = sum(n for _, n in blks)
            sched.append(dict(sb=s, w=w, pw=pw_list[w], col0=col,
                              blocks=blks, nch=nch))
            col += nch
    total_chunks = max(col, 1)

    per_core = [dict() for _ in range(ncore)]
    for c in range(ncore):
        idx_all = np.zeros((total_chunks * P,), np.int64)
        drel_all = np.full((total_chunks * P,), -1.0, np.float32)
        for ent in sched:
            off = ent["col0"] * P
            g = groups[c][ent["w"]]
            for b, nch in ent["blocks"]:
                if b in g:
                    r, dr = g[b]
                    idx_all[off:off + len(r)] = r
                    drel_all[off:off + len(r)] = dr
                off += nch * P
        per_core[c]["eidx"] = _wrap_idx(idx_all.astype(np.int16))
        per_core[c]["edrel"] = _colmajor_chunks(drel_all, total_chunks).astype(np.float16)

        sh = slice(c * shard, (c + 1) * shard)
        per_core[c]["dnsn"] = _colmajor_chunks(dstn_p[sh] * srcn_p[sh], sb_n)
        per_core[c]["dstn"] = _colmajor_chunks(dstn_p[sh], sb_n)
        per_core[c]["invd"] = (1.0 / dstn_p[sh]).astype(np.float16)[None, :]
        per_core[c]["srcn"] = _colmajor_chunks(srcn_p[sh], sb_n)

        fpad0 = np.zeros(shard, np.int64)
        fpad1 = np.full(shard, cfg.V, np.int64)
        nreal = max(0, min(shard, N - c * shard))
        fpad0[:nreal] = f0[c * shard: c * shard + nreal]
        fpad1[:nreal] = f1[c * shard: c * shard + nreal] + cfg.V
        per_core[c]["fidx"] = _wrap_idx(np.concatenate([fpad0, fpad1]).astype(np.int16))

        gid_n = np.full(shard, -1.0, np.float32)
        ginv_n = np.zeros(shard, np.float32)
        if nreal > 0:
            gids = graph_id[c * shard: c * shard + nreal]
            cnts = np.maximum(np.bincount(graph_id, minlength=cfg.B), 1.0)
            gid_n[:nreal] = gids
            ginv_n[:nreal] = 1.0 / cnts[gids]
        per_core[c]["gpind"] = np.stack([
            _colmajor_chunks(gid_n, sb_n),
            _colmajor_chunks(ginv_n, sb_n)]).astype(np.float16)

        selb = np.zeros((cfg.G_core, cfg.B), np.float16)
        for j in range(cfg.G_core):
            selb[j, c * cfg.G_core + j] = 1.0
        per_core[c]["gsel"] = np.ascontiguousarray(
            np.tile(selb.reshape(1, -1), (P, 1)))

    shared = dict(
        emb01=np.concatenate([emb0, emb1], 0).astype(np.float16),
        gcnw=gcn_w.astype(np.float16),
        gcnb=gcn_b.astype(np.float16).reshape(1, cfg.NL * cfg.D),
    )

    # ---- P-stage ----
    slots_pc = cfg.SLB * P
    g_ph, rows_g = cfg.node_phase_row(gni)
    g_w = rows_g // cfg.window
    g_rel = rows_g - g_w * cfg.window
    seg_core = gsi // slots_pc
    cnt_slots = np.bincount(gsi, minlength=cfg.B * cfg.L).astype(np.float32)

    pgroups = []
    for c in range(ncore):
        m = seg_core == c
        sl = gsi[m] - c * slots_pc
        per_w = []
        for (p, w) in pw_list:
            wm = (g_w[m] == w) & (g_ph[m] == p)
            per_w.append(_group_edges(g_rel[m][wm], (sl % P)[wm], (sl // P)[wm], cfg.SLB))
        pgroups.append(per_w)
    pcap = np.zeros((cfg.SLB, n_pw), np.int64)
    for c in range(ncore):
        for w in range(n_pw):
            for b, (r, _) in pgroups[c][w].items():
                pcap[b, w] = max(pcap[b, w], math.ceil(len(r) / P))
    psched = []
    col = 0
    for w in range(n_pw):
        blks = [(b, int(pcap[b, w])) for b in range(cfg.SLB) if pcap[b, w] > 0]
        nch = sum(n for _, n in blks)
        psched.append(dict(w=w, pw=pw_list[w], col0=col, blocks=blks, nch=nch))
        col += nch
    ptotal = max(col, 1)

    valid = (cnt_slots > 0).reshape(cfg.B, cfg.L)
    nvalid = valid.sum(1).astype(np.float32)
    sqd = math.sqrt(cfg.dh)
    Dd = cfg.D
    wq, wk, wv = ipw[:Dd], ipw[Dd:2 * Dd], ipw[2 * Dd:]
    bq, bk, bv = ipb[:Dd], ipb[Dd:2 * Dd], ipb[2 * Dd:]
    W1, W2 = out_w[:, :Dd], out_w[:, Dd:]
    Wc1 = W1 @ opw
    bc1 = W1 @ opb

    for c in range(ncore):
        pidx_all = np.zeros((ptotal * P,), np.int64)
        psrel_all = np.full((ptotal * P,), -1.0, np.float32)
        for ent in psched:
            w = ent["w"]
            off = ent["col0"] * P
            g = pgroups[c][w]
            for b, nch in ent["blocks"]:
                if b in g:
                    r, sr = g[b]
                    pidx_all[off:off + len(r)] = r
                    psrel_all[off:off + len(r)] = sr
                off += nch * P
        per_core[c]["pidx"] = _wrap_idx(pidx_all.astype(np.int16))
        per_core[c]["psrel"] = _colmajor_chunks(psrel_all, ptotal).astype(np.float16)
        ic = 1.0 / np.maximum(cnt_slots[c * slots_pc:(c + 1) * slots_pc], 1.0)
        per_core[c]["pinv"] = np.ascontiguousarray(ic[None, :]).astype(np.float32)

        gslc = slice(c * cfg.G_core, (c + 1) * cfg.G_core)
        mb = np.where(valid[gslc].reshape(-1), 0.0, -1e9).astype(np.float32)
        per_core[c]["maskb"] = np.ascontiguousarray(mb[None, :])
        vm = np.zeros((cfg.SLB, P, cfg.G_core), np.float32)
        for t in range(cfg.SLB):
            for p in range(P):
                sglob = t * P + p
                g_loc, l_loc = sglob // cfg.L, sglob % cfg.L
                if valid[c * cfg.G_core + g_loc, l_loc]:
                    vm[t, p, g_loc] = 1.0
        per_core[c]["vmask"] = np.ascontiguousarray(
            vm.transpose(1, 0, 2).reshape(P, cfg.SLB * cfg.G_core)).astype(np.float16)
        per_core[c]["bias2"] = np.ascontiguousarray(
            np.stack([nvalid[gslc], np.ones(cfg.G_core, np.float32)]))

    shared.update(
        pwqT=np.ascontiguousarray(wq.T / sqd).astype(np.float32),
        pwkT=np.ascontiguousarray(wk.T).astype(np.float32),
        pwvT=np.ascontiguousarray(wv.T).astype(np.float32),
        pbq=np.ascontiguousarray((bq / sqd)[:, None]).astype(np.float32),
        pbk=np.ascontiguousarray(bk[:, None]).astype(np.float32),
        pbv=np.ascontiguousarray(bv[None, :]).astype(np.float32),
        wc1T=np.ascontiguousarray(Wc1.T).astype(np.float32),
        w2T=np.ascontiguousarray(W2.T).astype(np.float32),
        bias2r=np.ascontiguousarray(np.stack([bc1, out_b])).astype(np.float32),
    )

    in_maps = []
    for c in range(ncore):
        d = dict(per_core[c])
        d.update(shared)
        in_maps.append(d)
    meta = dict(sched=sched, psched=psched, total_chunks=total_chunks, ptotal=ptotal)
    return in_maps, meta


# ----------------------------------------------------------------------------
def build_kernel(cfg: Cfg, meta, x, timing=False):
    sched, psched = meta["sched"], meta["psched"]
    total_chunks, ptotal = meta["total_chunks"], meta["ptotal"]
    sb_n, n_sb = cfg.shard_blocks, cfg.n_sb
    n_pw = len(sched) // n_sb
    shard = cfg.shard
    PH_R, PH_N, BP0 = cfg.ph_rows, cfg.ph_N, cfg.blk_ph0
    NL, D, B, Lq, G, SLB, OUT = cfg.NL, cfg.D, cfg.B, cfg.L, cfg.G_core, cfg.SLB, cfg.OUT
    dh, SBk = cfg.dh, cfg.SB
    max_nch = max([e["nch"] for e in sched] + [e["nch"] for e in psched] + [1])

    nc = bacc.Bacc("TRN2", target_bir_lowering=False, debug=False,
                   num_devices=1 if timing else cfg.n_cores)

    def param(name, dt):
        return nc.dram_tensor(name, list(x[name].shape), dt, kind="ExternalInput")

    eidx, edrel = param("eidx", I16), param("edrel", F16)
    dnsn_p, dstn_p = param("dnsn", F32), param("dstn", F32)
    invd_p, srcn_p = param("invd", F16), param("srcn", F32)
    fidx, gpind = param("fidx", I16), param("gpind", F16)
    emb01, gcnw, gcnb = param("emb01", F16), param("gcnw", F16), param("gcnb", F16)
    pidx, psrel_pp = param("pidx", I16), param("psrel", F16)
    pinv_p, maskb_p = param("pinv", F32), param("maskb", F32)
    vmask_p, bias2_p = param("vmask", F16), param("bias2", F32)
    pwqT, pwkT, pwvT = param("pwqT", F32), param("pwkT", F32), param("pwvT", F32)
    pbq, pbk, pbv = param("pbq", F32), param("pbk", F32), param("pbv", F32)
    wc1T_p, w2T_p = param("wc1T", F32), param("w2T", F32)
    bias2r_p, gsel_p = param("bias2r", F32), param("gsel", F16)
    out_ext = nc.dram_tensor("out", [G, OUT], F32, kind="ExternalOutput")

    rg = [list(range(cfg.n_cores))]

    with tile.TileContext(nc) as tc:
        with (
            tc.tile_pool(name="dram", bufs=1, space="DRAM") as dram,
            tc.tile_pool(name="res", bufs=1) as res,
            tc.tile_pool(name="io", bufs=4) as io,
            tc.tile_pool(name="blk", bufs=8) as blkp,
            tc.tile_pool(name="ps", bufs=2, space="PSUM") as psp,
        ):
            # ---------- resident constants ----------
            ident = res.tile([P, P], F16, tag="ident")
            make_identity(nc, ident[:])
            iota_i = res.tile([P, P], mybir.dt.int32, tag="iotai")
            nc.gpsimd.iota(iota_i[:], [[1, P]], channel_multiplier=0)
            iota_t = res.tile([P, P], F16, tag="iota")
            nc.vector.tensor_copy(iota_t[:], iota_i[:])
            iota3 = iota_t[:].rearrange("p (a f) -> p a f", a=1)

            drel_r = res.tile([P, total_chunks], F16, tag="drel")
            nc.sync.dma_start(drel_r[:], edrel[:, :])
            dnsn_r = res.tile([P, sb_n], F32, tag="dnsn")
            nc.sync.dma_start(dnsn_r[:], dnsn_p[:, :])
            dstn_r = res.tile([P, sb_n], F32, tag="dstnr")
            nc.sync.dma_start(dstn_r[:], dstn_p[:, :])
            srcn_r = res.tile([P, sb_n], F32, tag="srcnr")
            nc.sync.dma_start(srcn_r[:], srcn_p[:, :])
            invd_r = res.tile([1, shard], F16, tag="invd")
            nc.sync.dma_start(invd_r[:], invd_p[:, :])
            gcnw_r = res.tile([P, NL * D], F16, tag="gcnw")
            for l in range(NL):
                nc.sync.dma_start(gcnw_r[:, l * D:(l + 1) * D], gcnw[l, :, :])
            gcnb_r = res.tile([1, NL * D], F16, tag="gcnb")
            nc.sync.dma_start(gcnb_r[:], gcnb[:, :])
            gpind_r = res.tile([P, sb_n * B], F16, tag="gpind")
            gid_r = res.tile([P, sb_n], F16, tag="gid")
            nc.sync.dma_start(gid_r[:], gpind[0, :, :])
            ginv_r = res.tile([P, sb_n], F16, tag="ginv")
            nc.sync.dma_start(ginv_r[:], gpind[1, :, :])
            gpind3 = gpind_r[:].rearrange("p (c b) -> p c b", b=B)
            # gpind[p, c, g] = (gid[p,c] == g) * ginv[p,c]  (built on device
            # from 2 compact rows instead of shipping the 1.6MB one-hot)
            nc.vector.tensor_tensor(
                out=gpind3, in0=iota_t[:, :B].rearrange("p (a f) -> p a f", a=1)
                    .broadcast_to((P, sb_n, B)),
                in1=gid_r[:].broadcast_to((P, sb_n, B)),
                op=mybir.AluOpType.is_equal)
            nc.vector.tensor_tensor(
                out=gpind3, in0=gpind3,
                in1=ginv_r[:].broadcast_to((P, sb_n, B)),
                op=mybir.AluOpType.mult)
            ones1 = res.tile([1, P], F32, tag="ones1")
            nc.vector.memset(ones1[:], 1.0)
            zed = res.tile([1, P], F16, tag="zed")
            nc.vector.memset(zed[:], 0.0)

            g_p = [[dram.tile([PH_N[p], D], F16, tag=f"gfull{l}p{p}",
                              name=f"gfull{l}p{p}",
                              addr_space="Shared" if (not timing and cfg.n_cores > 4) else "Local")
                    for p in range(2)]
                   for l in range(NL + 1)]
            bounce = [[dram.tile([PH_R[p], D], F16, tag=f"bounce{l}p{p}",
                                 name=f"bounce{l}p{p}") for p in range(2)]
                      for l in range(NL + 1)]

            def bounce_rows(l, b):
                """(dram tile, row0) for dst block b of layer-l output."""
                if b < BP0:
                    return bounce[l][0], b * P
                return bounce[l][1], (b - BP0) * P

            agg16_r = res.tile([P, n_sb * SBk * P], F16, tag="agg16")
            zblk = res.tile([P, D], F16, tag="zblk")
            nc.vector.memset(zblk[:], 0.0)
            for l in range(NL + 1):
                nc.sync.dma_start(bounce[l][1][PH_R[1] - P:, :], zblk[:])

            def allgather(l, ph):
                dst_t = g_p[l][ph]
                if timing:
                    nc.sync.dma_start(dst_t[0:PH_R[ph], :], bounce[l][ph][:, :])
                    return
                nc.gpsimd.collective_compute(
                    "AllGather", mybir.AluOpType.bypass, replica_groups=rg,
                    ins=[bounce[l][ph].opt()], outs=[dst_t.opt()])

            def load_idx(idx_t, src_slice, ncols):
                """DMA [16, ncols] idx rows into [128, ncols] SBUF, replicated
                across the 8 Q7 partition groups via a step-0 source AP."""
                nc.sync.dma_start(
                    idx_t[:, :ncols],
                    src_slice.rearrange("(a r) n -> a r n", a=1).broadcast_to(
                        (8, 16, ncols)))

            def gather_rows(out3, src_ap, idx_tile, nch):
                """dma_gather split into <=8-chunk (1024-idx) instructions to
                bound per-instruction descriptor-ring usage."""
                for o in range(0, nch, 8):
                    n = min(8, nch - o)
                    nc.gpsimd.dma_gather(
                        out_ap=out3[:, o:o + n, :], in_ap=src_ap,
                        idxs_ap=idx_tile[:, o * 8:(o + n) * 8],
                        num_idxs=n * P, num_idxs_reg=n * P,
                        elem_size=D, single_packet=False)

            def wmat_tail(l_w, s_t, b, dest_l):
                """transpose s_t, multiply by gcn_w[l_w], write block b of the
                layer-dest_l output to its phase bounce."""
                tp = psp.tile([P, P], F16, tag="tp")
                nc.tensor.transpose(out=tp[:], in_=s_t[:], identity=ident[:])
                sT = blkp.tile([P, P], F16, tag="sT")
                nc.scalar.copy(sT[:], tp[:])
                gp = psp.tile([P, 256], F32, tag="gp", bufs=1)
                nc.tensor.matmul(out=gp[:, :D], lhsT=sT[:],
                                 rhs=gcnw_r[:, l_w * D:(l_w + 1) * D],
                                 start=True, stop=True)
                g_t = blkp.tile([P, D], F16, tag="g")
                nc.scalar.copy(g_t[:], gp[:, :D])
                dest, r0 = bounce_rows(dest_l, b)
                nc.sync.dma_start(dest[r0:r0 + P, :], g_t[:])

            # ---------- E-stage ----------
            fidx_t = io.tile([P, 2 * shard // 16], I16, tag="fidx", bufs=1)
            load_idx(fidx_t, fidx[:, :], 2 * shard // 16)
            hv0 = io.tile([P, sb_n, D], F16, tag="hv0", bufs=1)
            for o in range(0, sb_n, 16):
                n = min(16, sb_n - o)
                fa = io.tile([P, 16, D], F16, tag="fa", bufs=2)
                gather_rows(fa[:, :n, :], emb01[:, :],
                            fidx_t[:, o * 8:(o + n) * 8], n)
                fb = io.tile([P, 16, D], F16, tag="fb", bufs=2)
                gather_rows(fb[:, :n, :], emb01[:, :],
                            fidx_t[:, (sb_n + o) * 8:(sb_n + o + n) * 8], n)
                nc.vector.tensor_tensor(out=hv0[:, o:o + n, :],
                                        in0=fa[:, :n, :], in1=fb[:, :n, :],
                                        op=mybir.AluOpType.add)
            for b in range(sb_n):
                s0 = blkp.tile([P, D], F16, tag="s")
                nc.vector.tensor_scalar_mul(s0[:], hv0[:, b, :], srcn_r[:, b:b + 1])
                wmat_tail(0, s0, b, 0)
                if b == BP0 - 1:
                    allgather(0, 0)
            allgather(0, 1)

            # ---------- A-layers ----------
            def entry_work(l, ent, aggs, first, remaining):
                """gather + one-hot + accumulate matmuls for one sched entry.
                remaining: per-block chunk countdown across entries; the
                block's psum group closes when it hits zero (None = never)."""
                nch = ent["nch"]
                ph, w = ent["pw"]
                idx_t = io.tile([P, max_nch * 8], I16, tag="idx", bufs=8)
                load_idx(idx_t, eidx[:, ent["col0"] * 8:(ent["col0"] + nch) * 8],
                         nch * 8)
                msgs = io.tile([P, max_nch, D], F16, tag="msgs")
                wlo = w * cfg.window
                whi = min(wlo + cfg.window, PH_N[ph])
                gather_rows(msgs[:], g_p[l][ph][wlo:whi, :], idx_t[:], nch)
                oh = io.tile([P, max_nch, D], F16, tag="oh")
                nc.vector.tensor_tensor(
                    out=oh[:, :nch, :],
                    in0=iota3.broadcast_to((P, nch, P)),
                    in1=drel_r[:, ent["col0"]:ent["col0"] + nch]
                        .broadcast_to((P, nch, P)),
                    op=mybir.AluOpType.is_equal)
                k = 0
                for b, bn in ent["blocks"]:
                    for _ in range(bn):
                        if remaining is not None:
                            remaining[b] -= 1
                        nc.tensor.matmul(
                            out=aggs[b][:], lhsT=oh[:, k, :],
                            rhs=msgs[:, k, :],
                            start=first[b],
                            stop=(remaining is not None and remaining[b] == 0))
                        first[b] = False
                        k += 1

            for l in range(NL):
                last = l == NL - 1
                if last:
                    gpool_ps = psp.tile([P, B], F32, tag="sm", bufs=1)
                # pass 1: all phase-0 groups, partial agg -> fp16 SBUF.
                # pass 2: phase-1 groups + bias + finish + tails.  This keeps
                # every phase-1 gather (which waits on the in-flight phase-1
                # AllGather) behind a full half-layer of phase-0 work on the
                # gpsimd queue, hiding the collective.
                has_p0 = {}
                for s in range(n_sb):
                    blocks = list(range(s * SBk, min((s + 1) * SBk, sb_n)))
                    ents = [sched[s * n_pw + wi] for wi in range(n_pw)]
                    p0 = [e for e in ents if e["pw"][0] == 0 and e["nch"] > 0]
                    for b in blocks:
                        has_p0[b] = any(b == bb for e in p0 for bb, _ in e["blocks"])
                    if not p0:
                        continue
                    aggs = {b: psp.tile([P, P], F32, tag=f"agg{b - s * SBk}",
                                        bufs=1, name=f"agg{b - s * SBk}")
                            for b in blocks}
                    first = {b: True for b in blocks}
                    remaining = {b: sum(bn for e in p0 for bb, bn in e["blocks"]
                                        if bb == b) for b in blocks}
                    for e in p0:
                        entry_work(l, e, aggs, first, remaining)
                    for b in blocks:
                        if has_p0[b]:
                            nc.vector.tensor_copy(
                                agg16_r[:, b * P:(b + 1) * P], aggs[b][:])
                for s in range(n_sb):
                    blocks = list(range(s * SBk, min((s + 1) * SBk, sb_n)))
                    ents = [sched[s * n_pw + wi] for wi in range(n_pw)]
                    p1 = [e for e in ents if e["pw"][0] == 1 and e["nch"] > 0]
                    aggs = {b: psp.tile([P, P], F32, tag=f"agg{b - s * SBk}",
                                        bufs=1, name=f"agg{b - s * SBk}")
                            for b in blocks}
                    first = {b: True for b in blocks}
                    for e in p1:
                        entry_work(l, e, aggs, first, None)
                    for b in blocks:
                        nc.tensor.matmul(
                            out=aggs[b][:],
                            lhsT=invd_r[0:1, b * P:(b + 1) * P],
                            rhs=gcnb_r[0:1, l * D:(l + 1) * D],
                            start=first[b], stop=not has_p0[b])
                        if has_p0[b]:
                            # re-add the phase-0 partial on the PE: identity.T
                            # @ agg16 accumulates it into the open psum group
                            nc.tensor.matmul(
                                out=aggs[b][:], lhsT=ident[:],
                                rhs=agg16_r[:, b * P:(b + 1) * P],
                                start=False, stop=True)
                        fin_in = aggs[b]
                        s_t = blkp.tile([P, D], F16, tag="s")
                        scal = dstn_r if last else dnsn_r
                        nc.scalar.activation(
                            s_t[:], fin_in[:], mybir.ActivationFunctionType.Relu,
                            scale=scal[:, b:b + 1])
                        if not last:
                            wmat_tail(l + 1, s_t, b, l + 1)
                        else:
                            dest, r0 = bounce_rows(NL, b)
                            nc.sync.dma_start(dest[r0:r0 + P, :], s_t[:])
                            nc.tensor.matmul(
                                out=gpool_ps[:], lhsT=s_t[:], rhs=gpind3[:, b, :],
                                start=(b == 0), stop=(b == sb_n - 1))
                    if s == cfg.sb_ph0 - 1:
                        allgather(l + 1, 0)
                if last:
                    gpool_s = blkp.tile([P, B], F16, tag="gpool_s")
                    nc.vector.tensor_copy(gpool_s[:], gpool_ps[:])
                    nc.sync.dma_start(bounce[NL][1][PH_R[1] - P:, 0:B], gpool_s[:])
                allgather(l + 1, 1)

            # ---------- P-stage: group pool ----------
            psrel_r = res.tile([P, ptotal], F16, tag="psrel")
            nc.sync.dma_start(psrel_r[:], psrel_pp[:, :])
            mean_ps = [psp.tile([P, P], F32, tag=f"agg{t}", bufs=1,
                                name=f"mean{t}") for t in range(SLB)]
            p_first = [True] * SLB
            for ent in psched:
                nch = ent["nch"]
                if nch == 0:
                    continue
                idx_t = io.tile([P, max_nch * 8], I16, tag="idx", bufs=8)
                load_idx(idx_t, pidx[:, ent["col0"] * 8:(ent["col0"] + nch) * 8],
                         nch * 8)
                msgs = io.tile([P, max_nch, D], F16, tag="msgs")
                ph, w = ent["pw"]
                wlo = w * cfg.window
                whi = min(wlo + cfg.window, PH_N[ph])
                gather_rows(msgs[:], g_p[NL][ph][wlo:whi, :], idx_t[:], nch)
                oh = io.tile([P, max_nch, D], F16, tag="oh")
                nc.vector.tensor_tensor(
                    out=oh[:, :nch, :],
                    in0=iota3.broadcast_to((P, nch, P)),
                    in1=psrel_r[:, ent["col0"]:ent["col0"] + nch]
                        .broadcast_to((P, nch, P)),
                    op=mybir.AluOpType.is_equal)
                k = 0
                for b, bn in ent["blocks"]:
                    for _ in range(bn):
                        nc.tensor.matmul(
                            out=mean_ps[b][:],
                            lhsT=msgs[:, k, :], rhs=oh[:, k, :],
                            start=p_first[b], stop=False)
                        p_first[b] = False
                        k += 1
            pinv_r = res.tile([P, SLB * P], F32, tag="pinv")
            nc.sync.dma_start(
                pinv_r[:],
                pinv_p[:, :].rearrange("(a r) n -> a r n", a=1)
                    .broadcast_to((P, 1, SLB * P)))
            meansT = res.tile([P, SLB * P], F32, tag="meansT")
            for t in range(SLB):
                nc.tensor.matmul(out=mean_ps[t][:],
                                 lhsT=zed[0:1, :], rhs=zed[0:1, :],
                                 start=p_first[t], stop=True)
                nc.vector.tensor_tensor(
                    out=meansT[:, t * P:(t + 1) * P], in0=mean_ps[t][:],
                    in1=pinv_r[:, t * P:(t + 1) * P], op=mybir.AluOpType.mult)

            # ---------- attention ----------
            wq_r = res.tile([P, P], F32, tag="wq")
            nc.sync.dma_start(wq_r[:], pwqT[:, :])
            wk_r = res.tile([P, P], F32, tag="wk")
            nc.sync.dma_start(wk_r[:], pwkT[:, :])
            wv_r = res.tile([P, P], F32, tag="wv")
            nc.sync.dma_start(wv_r[:], pwvT[:, :])
            bq_r = res.tile([P, 1], F32, tag="bq")
            nc.sync.dma_start(bq_r[:], pbq[:, :])
            bk_r = res.tile([P, 1], F32, tag="bk")
            nc.sync.dma_start(bk_r[:], pbk[:, :])
            bv_r = res.tile([1, P], F32, tag="bv")
            nc.sync.dma_start(bv_r[:], pbv[:, :])

            SLOTS = SLB * P
            q_ps = psp.tile([P, 256], F32, tag="gp", bufs=1)
            k_ps = psp.tile([P, 256], F32, tag="gp", bufs=1)
            for t in range(SLB):
                nc.tensor.matmul(out=q_ps[:, t * P:(t + 1) * P], lhsT=wq_r[:],
                                 rhs=meansT[:, t * P:(t + 1) * P], start=True, stop=True)
                nc.tensor.matmul(out=k_ps[:, t * P:(t + 1) * P], lhsT=wk_r[:],
                                 rhs=meansT[:, t * P:(t + 1) * P], start=True, stop=True)
            qT = res.tile([P, SLOTS], F32, tag="qT")
            kT = res.tile([P, SLOTS], F32, tag="kT")
            nc.vector.tensor_scalar_add(qT[:], q_ps[:, :SLOTS], bq_r[:, 0:1])
            nc.vector.tensor_scalar_add(kT[:], k_ps[:, :SLOTS], bk_r[:, 0:1])

            S_ps = psp.tile([P, 256], F32, tag="gp", bufs=1)
            for g in range(G):
                for h in range(cfg.H):
                    hp, gp_ = h * dh, g * Lq
                    nc.tensor.matmul(
                        out=S_ps[hp:hp + dh, gp_:gp_ + Lq],
                        lhsT=qT[hp:hp + dh, gp_:gp_ + Lq],
                        rhs=kT[hp:hp + dh, gp_:gp_ + Lq],
                        start=True, stop=True, tile_position=(hp, hp))
            maskb_r = res.tile([P, SLOTS], F32, tag="maskb")
            nc.sync.dma_start(
                maskb_r[:],
                maskb_p[:, :].rearrange("(a r) n -> a r n", a=1)
                    .broadcast_to((P, 1, SLOTS)))
            Sm = res.tile([P, SLOTS], F32, tag="Sm")
            nc.vector.tensor_tensor(out=Sm[:], in0=S_ps[:, :SLOTS], in1=maskb_r[:],
                                    op=mybir.AluOpType.add)
            Sm3 = Sm[:].rearrange("p (g l) -> p g l", l=Lq)
            rmax = res.tile([P, G], F32, tag="rmax")
            nc.vector.tensor_reduce(out=rmax[:], in_=Sm3, axis=mybir.AxisListType.X,
                                    op=mybir.AluOpType.max)
            Sc = res.tile([P, SLOTS], F32, tag="Sc")
            nc.vector.tensor_tensor(out=Sc[:].rearrange("p (g l) -> p g l", l=Lq),
                                    in0=Sm3, in1=rmax[:].broadcast_to((P, G, Lq)),
                                    op=mybir.AluOpType.subtract)
            Se = res.tile([P, SLOTS], F32, tag="Se")
            nc.scalar.activation(Se[:], Sc[:], mybir.ActivationFunctionType.Exp)
            rsum = res.tile([P, G], F32, tag="rsum")
            nc.vector.tensor_reduce(out=rsum[:],
                                    in_=Se[:].rearrange("p (g l) -> p g l", l=Lq),
                                    axis=mybir.AxisListType.X, op=mybir.AluOpType.add)
            rinv = res.tile([P, G], F32, tag="rinv")
            nc.vector.reciprocal(rinv[:], rsum[:])
            attn = res.tile([P, SLOTS], F16, tag="attn")
            nc.vector.tensor_tensor(out=attn[:].rearrange("p (g l) -> p g l", l=Lq),
                                    in0=Se[:].rearrange("p (g l) -> p g l", l=Lq),
                                    in1=rinv[:].broadcast_to((P, G, Lq)),
                                    op=mybir.AluOpType.mult)

            vmask_r = res.tile([P, SLB * G], F16, tag="vmask")
            nc.sync.dma_start(vmask_r[:], vmask_p[:, :])
            omT_ps = psp.tile([P, B], F32, tag="sm", bufs=1)
            for t in range(SLB):
                aT_ps = psp.tile([P, P], F16, tag="tp")
                nc.tensor.transpose(out=aT_ps[:], in_=attn[:, t * P:(t + 1) * P],
                                    identity=ident[:])
                aT = blkp.tile([P, P], F16, tag="sT")
                nc.vector.tensor_copy(aT[:], aT_ps[:])
                v_ps = psp.tile([P, P], F32, tag="tp")
                nc.tensor.matmul(out=v_ps[:], lhsT=meansT[:, t * P:(t + 1) * P],
                                 rhs=wv_r[:], start=True, stop=False)
                nc.tensor.matmul(out=v_ps[:], lhsT=ones1[0:1, :], rhs=bv_r[0:1, :],
                                 start=False, stop=True)
                v_s = blkp.tile([P, P], F16, tag="s")
                nc.vector.tensor_copy(v_s[:], v_ps[:])
                o_ps = psp.tile([P, P], F32, tag="tp")
                for gi in range(P // Lq):
                    gp_ = gi * Lq
                    for h in range(cfg.H):
                        hp = h * dh
                        nc.tensor.matmul(
                            out=o_ps[gp_:gp_ + Lq, hp:hp + dh],
                            lhsT=aT[gp_:gp_ + Lq, hp:hp + dh],
                            rhs=v_s[gp_:gp_ + Lq, hp:hp + dh],
                            start=True, stop=True, tile_position=(gp_, gp_))
                o_s = blkp.tile([P, P], F16, tag="g")
                nc.vector.tensor_copy(o_s[:], o_ps[:])
                nc.tensor.matmul(
                    out=omT_ps[:, :G], lhsT=o_s[:],
                    rhs=vmask_r[:, t * G:(t + 1) * G],
                    start=(t == 0), stop=(t == SLB - 1))
            omT = res.tile([P, G], F32, tag="omTs")
            nc.vector.tensor_copy(omT[:], omT_ps[:, :G])

            # ---------- whole-graph pool: sum the per-core partials ----------
            poolT = res.tile([P, B], F16, tag="poolT")
            part0 = blkp.tile([P, B], F16, tag="part")
            nc.sync.dma_start(part0[:], g_p[NL][1][PH_R[1] - P:PH_R[1], 0:B])
            nc.vector.tensor_copy(poolT[:], part0[:])
            for c2 in range(1, cfg.n_cores):
                part2 = blkp.tile([P, B], F16, tag="part")
                nc.sync.dma_start(
                    part2[:],
                    g_p[NL][1][(c2 + 1) * PH_R[1] - P:(c2 + 1) * PH_R[1], 0:B])
                nc.vector.tensor_tensor(out=poolT[:], in0=poolT[:], in1=part2[:],
                                        op=mybir.AluOpType.add)

            # ---------- final linear ----------
            wc1_r = res.tile([P, OUT], F32, tag="wc1")
            nc.sync.dma_start(wc1_r[:], wc1T_p[:, :])
            w2_r = res.tile([P, OUT], F32, tag="w2")
            nc.sync.dma_start(w2_r[:], w2T_p[:, :])
            b2l_r = res.tile([2, G], F32, tag="b2l")
            nc.sync.dma_start(b2l_r[:], bias2_p[:, :])
            b2r_r = res.tile([2, OUT], F32, tag="b2r")
            nc.sync.dma_start(b2r_r[:], bias2r_p[:, :])
            gsel_r = res.tile([P, G * B], F16, tag="gsel")
            nc.sync.dma_start(gsel_r[:], gsel_p[:, :])
            ptmp = res.tile([P, G * B], F32, tag="ptmp")
            nc.vector.tensor_tensor(
                out=ptmp[:].rearrange("p (g b) -> p g b", b=B),
                in0=poolT[:].rearrange("p (a b) -> p a b", a=1)
                    .broadcast_to((P, G, B)),
                in1=gsel_r[:].rearrange("p (g b) -> p g b", b=B),
                op=mybir.AluOpType.mult)
            poolsel = res.tile([P, G], F32, tag="poolsels")
            nc.vector.tensor_reduce(
                out=poolsel[:], in_=ptmp[:].rearrange("p (g b) -> p g b", b=B),
                axis=mybir.AxisListType.X, op=mybir.AluOpType.add)

            out_ps = psp.tile([G, OUT], F32, tag="sm", bufs=1)
            nc.tensor.matmul(out=out_ps[:], lhsT=omT[:], rhs=wc1_r[:],
                             start=True, stop=False)
            nc.tensor.matmul(out=out_ps[:], lhsT=poolsel[:], rhs=w2_r[:],
                             start=False, stop=False)
            nc.tensor.matmul(out=out_ps[:], lhsT=b2l_r[:], rhs=b2r_r[:],
                             start=False, stop=True)
            out_s = res.tile([G, OUT], F32, tag="out_s")
            nc.vector.tensor_copy(out_s[:], out_ps[:])
            nc.sync.dma_start(out_ext[:, :], out_s[:])

    nc.compile()
    return nc


# ============================================================================
# harness entry point
# ============================================================================
import os as _os
from concourse.bass_utils import run_bass_kernel_spmd

_BUILD_CACHE = {}
LAST_RESULTS = None


def kernel(**inputs):
    cfg = Cfg()
    assert int(inputs.get("num_graphs", cfg.B)) == cfg.B
    assert int(inputs.get("max_len", cfg.L)) == cfg.L
    in_maps, meta = preprocess(cfg, inputs)
    key = (meta["total_chunks"], meta["ptotal"],
           tuple(e["nch"] for e in meta["sched"]),
           tuple(e["nch"] for e in meta["psched"]))
    if key not in _BUILD_CACHE:
        _BUILD_CACHE.clear()
        _BUILD_CACHE[key] = build_kernel(cfg, meta, in_maps[0])
    nc = _BUILD_CACHE[key]
    res = run_bass_kernel_spmd(nc, in_maps, core_ids=list(range(cfg.n_cores)))
    global LAST_RESULTS
    LAST_RESULTS = res
    out = np.concatenate([r["out"] for r in res.results], 0)
    return out[:cfg.B].astype(np.float32)

